# revision 13
# baseline (speedup 1.0000x reference)
"""Trainium2 Bass kernel for nn_DFlashDraftModel (dense draft transformer).

Sharding: tensor-parallel over heads across 8 cores (2 Q heads + 1 KV head
per core), MLP columns/rows 8-way, fc (target_hidden projection) row-sharded
with one AllGather, 2 AllReduces per layer for the (tiny) hidden stream.

On-device layout is feature-major ("transposed"): activations are stored as
[feature_partition, token] so every matmul consumes weights [in, out] directly
as the stationary lhsT operand and no activation transposes are needed except
V (PE-transposed per 128-row tile for the PV matmul).

The fc output is AllGathered RAW (scaled only by hidden_norm_w): the
per-token rstd of the hidden RMS norm cancels inside the per-head K norm
(RMS norm is scale-invariant per column), and enters the V path through the
softmax instead: exp-bias ln(rstd) scales the attention weights, and a
1/rstd column replaces the ones vector in the denominator sum. This lets the
first AllGather half fire at fc midpoint, decoupled from the fc stats tail.

build_program(reps=N) repeats the whole forward pass N times inside one
NEFF (tile names suffixed per rep, tags shared so SBUF slots rotate) — used
only for amplified timing; the deliverable path uses reps=1.
"""

import numpy as np
import ml_dtypes

import concourse.bass as bass
import concourse.tile as tile
from concourse import bacc, mybir
from concourse.bass_utils import run_bass_kernel_spmd
from concourse.masks import make_identity
from contextlib import ExitStack

AF = mybir.ActivationFunctionType
ALU = mybir.AluOpType
F32 = mybir.dt.float32
BF16 = mybir.dt.bfloat16
BF = ml_dtypes.bfloat16

# model dims
B, Q, CTX, L, H = 2, 32, 2048, 4, 2048
NH, NKV, HD, INTER = 16, 8, 128, 6144
KV = CTX + Q           # 2080
KT = H // 128          # 16 feature tiles
FT = 8192 // 128       # 64 fc contraction tiles
IT = (INTER // 8) // 128  # 6 inter tiles per core
XC = B * Q             # 64 hidden-stream columns
COLS = B * KV          # 4160 kv columns
RWS = (B * CTX) // 8   # 512 fc rows per core
NCORES = 8
EPS = 1e-6
THETA = 1000000.0
SCALE = HD ** -0.5
RG = [list(range(NCORES))]

TRACE = False
FAKE_COLL = False  # replace collectives with local DMAs (TimelineSim analysis)
_CACHE = {}


def _bcol(b, j):
    """column offset/width in the [*, 4160] kv panel for batch b, n-tile j"""
    off = b * KV + j * 512
    w = 512 if j < 4 else KV - CTX  # tail tile = the 32 x-columns
    return off, w


class _RPool:
    """Pool proxy: suffixes tile names per rep, pinning tags to base names
    so SBUF/DRAM slots rotate across reps. Identity when reps == 1."""

    def __init__(self, pool, rp, reps):
        self._pool = pool
        self._rp = rp
        self._reps = reps

    def tile(self, shape, dtype, name=None, tag=None, **kw):
        if self._reps > 1:
            tag = tag if tag is not None else name
            name = f"{name}_R{self._rp}"
        return self._pool.tile(shape, dtype, name=name, tag=tag, **kw)


def build_program(reps=1):
    nc = bacc.Bacc("TRN2", target_bir_lowering=False, debug=False,
                   enable_asserts=True, num_devices=NCORES)

    # ---------------- I/O ----------------
    thT_h = nc.dram_tensor("thT", [8192, RWS], BF16, kind="ExternalInput")
    fcw_h = nc.dram_tensor("fcw", [16, 128, 8192], BF16, kind="ExternalInput")
    hT0_h = nc.dram_tensor("hT0", [H, XC], F32, kind="ExternalInput")
    wq_h = nc.dram_tensor("wq", [L, 2, 128, 2048], BF16, kind="ExternalInput")
    wk_h = nc.dram_tensor("wk", [L, 128, 2048], BF16, kind="ExternalInput")
    wv_h = nc.dram_tensor("wv", [L, 128, 2048], BF16, kind="ExternalInput")
    wo_h = nc.dram_tensor("wo", [L, 16, 128, 256], BF16, kind="ExternalInput")
    gw_h = nc.dram_tensor("gw", [L, 6, 128, 2048], BF16, kind="ExternalInput")
    uw_h = nc.dram_tensor("uw", [L, 6, 128, 2048], BF16, kind="ExternalInput")
    dw_h = nc.dram_tensor("dw", [L, 16, 128, 768], BF16, kind="ExternalInput")
    csk_h = nc.dram_tensor("csk", [128, COLS], BF16, kind="ExternalInput")
    csn_h = nc.dram_tensor("csn", [128, COLS], BF16, kind="ExternalInput")
    csq_h = nc.dram_tensor("csq", [128, XC], BF16, kind="ExternalInput")
    csqn_h = nc.dram_tensor("csqn", [128, XC], BF16, kind="ExternalInput")
    ln1_h = nc.dram_tensor("ln1w", [128, L * KT], F32, kind="ExternalInput")
    ln2_h = nc.dram_tensor("ln2w", [128, L * KT], F32, kind="ExternalInput")
    hnw_h = nc.dram_tensor("hnw", [128, KT], F32, kind="ExternalInput")
    fnw_h = nc.dram_tensor("fnw", [128, KT], F32, kind="ExternalInput")
    qnw_h = nc.dram_tensor("qnw", [128, L], F32, kind="ExternalInput")
    knw_h = nc.dram_tensor("knw", [128, L], F32, kind="ExternalInput")
    outT_h = nc.dram_tensor("outT", [H, XC], F32, kind="ExternalOutput")

    with tile.TileContext(nc) as tc, ExitStack() as ctx:
        # ---------------- pools ----------------
        pre0 = ctx.enter_context(tc.tile_pool(name="pre", bufs=1))
        dram0 = ctx.enter_context(tc.tile_pool(name="dram", bufs=1,
                                               space="DRAM"))
        arp0 = ctx.enter_context(tc.tile_pool(name="arp", bufs=2,
                                              space="DRAM"))
        stats0 = ctx.enter_context(tc.tile_pool(name="stats", bufs=1))
        temps0 = ctx.enter_context(tc.tile_pool(name="temps", bufs=2))
        # psum pools: 2 + 2 + 3 + 1 = 8 banks
        mmp0 = ctx.enter_context(tc.tile_pool(name="mmp", bufs=2,
                                              space="PSUM"))
        mm640 = ctx.enter_context(tc.tile_pool(name="mm64", bufs=3,
                                               space="PSUM"))
        scp0 = ctx.enter_context(tc.tile_pool(name="scp", bufs=2,
                                              space="PSUM"))
        ssqp0 = ctx.enter_context(tc.tile_pool(name="ssqp", bufs=1,
                                               space="PSUM"))

        # ---------------- constants (once) ----------------
        ones_bf = pre0.tile([128, 1], BF16, name="ones_bf")
        nc.vector.memset(ones_bf, 1.0)
        zb = pre0.tile([128, 1], F32, name="zb")
        nc.vector.memset(zb, 0.0)
        epsb = pre0.tile([1, 1], F32, name="epsb")
        nc.vector.memset(epsb, EPS)
        ident = pre0.tile([128, 128], BF16, name="ident")
        make_identity(nc, ident)
        csq = pre0.tile([128, XC], BF16, name="csq")
        csqn = pre0.tile([128, XC], BF16, name="csqn")
        ln1 = pre0.tile([128, L * KT], F32, name="ln1")
        ln2 = pre0.tile([128, L * KT], F32, name="ln2")
        hnw = pre0.tile([128, KT], F32, name="hnw")
        nc.sync.dma_start(out=hnw, in_=hnw_h.ap())
        fnw = pre0.tile([128, KT], F32, name="fnw")
        qnw = pre0.tile([128, L], F32, name="qnw")
        knw = pre0.tile([128, L], F32, name="knw")

        HK = KT // 2
        persist = {}

        def body(rp):
            pre = _RPool(pre0, rp, reps)
            dram = _RPool(dram0, rp, reps)
            arp = _RPool(arp0, rp, reps)
            stats = _RPool(stats0, rp, reps)
            temps = _RPool(temps0, rp, reps)
            mmp = _RPool(mmp0, rp, reps)
            mm64 = _RPool(mm640, rp, reps)
            scp = _RPool(scp0, rp, reps)
            ssqp = _RPool(ssqp0, rp, reps)

            hT = pre.tile([128, KT * XC], F32, name="hT")  # residual stream

            # th allgather split into two feature halves so the first
            # collective fires at fc midpoint.
            th_loc2 = [dram.tile([H // 2, RWS], BF16, name=f"th_loc{i}")
                       for i in range(2)]
            th_all2 = [dram.tile([NCORES * H // 2, RWS], BF16,
                                 name=f"th_all{i}", addr_space="Shared")
                       for i in range(2)]
            # per-token rstd sideband (gathered as rms = 1/rstd)
            rl_loc = dram.tile([1, RWS], F32, name="rl_loc")
            rl_all = dram.tile([NCORES, RWS], F32, name="rl_all",
                               addr_space="Shared")

            def coll(kind, op, in_t, out_t):
                if FAKE_COLL:
                    nc.sync.dma_start(out=out_t[0:in_t.shape[0], :], in_=in_t)
                else:
                    nc.gpsimd.collective_compute(
                        kind, op, replica_groups=RG,
                        ins=[in_t.opt()], outs=[out_t.opt()])

            # ----- helper: column RMS stats -> broadcast 1/rms [128, w] -----
            def rms_bcast(srcs, w, div, nm):
                ssq = ssqp.tile([1, 512], F32, name=f"ssq_{nm}", tag="ssq")
                n = len(srcs)
                for i, ap in enumerate(srcs):
                    sq = temps.tile([128, w], BF16, name=f"sq_{nm}_{i}",
                                    tag="sq512", bufs=1)
                    nc.vector.tensor_mul(sq, ap, ap)
                    nc.tensor.matmul(ssq[:, :w], ones_bf[:, 0:1], sq,
                                     start=(i == 0), stop=(i == n - 1))
                nc.scalar.activation(ssq[:, :w], ssq[:, :w], AF.Sqrt,
                                     bias=epsb[:, 0:1], scale=1.0 / div)
                rc = stats.tile([1, w], F32, name=f"rc_{nm}", tag="rs2")
                nc.vector.reciprocal(rc, ssq[:, :w])
                rb = temps.tile([128, w], F32, name=f"rb_{nm}", tag="rstdb",
                                bufs=1)
                nc.gpsimd.partition_broadcast(rb, rc)
                return rb

            # ----- helper: rope. cs/sn are cos/sin duplicated across halves.
            def rope(src, dst, cs, sn, nm):
                w = src.shape[1]
                srot = temps.tile([128, w], BF16, name=f"srot_{nm}",
                                  tag="srot", bufs=1)
                # rotate-half copies run on the idle GPSIMD engine
                nc.gpsimd.tensor_scalar_mul(srot[0:64, :], src[64:128, :],
                                            -1.0)
                nc.gpsimd.tensor_copy(srot[64:128, :], src[0:64, :])
                rt = temps.tile([128, w], BF16, name=f"rt_{nm}", tag="rtmp",
                                bufs=1)
                nc.vector.tensor_mul(rt, srot, sn)
                nc.vector.tensor_mul(dst, src, cs)
                nc.vector.tensor_add(dst, dst, rt)

            def spread_half(half, dst3):
                """scatter th_all2[half] into the SBUF panel [128, 8, B*CTX]"""
                for r in range(NCORES):
                    b, j = divmod(r, 4)
                    eng = nc.sync if r % 2 == 0 else nc.scalar
                    out3 = dst3[:, :, b * CTX + j * 512:
                                b * CTX + (j + 1) * 512]
                    eng.dma_start(
                        out=out3,
                        in_=th_all2[half][r * (H // 2):(r + 1) * (H // 2), :]
                        .rearrange("(kh p) n -> p kh n", p=128))

            # ---------------- phase 1: fc matmul (raw, hnw-scaled) ----------
            with tc.tile_pool(name="fcp", bufs=1) as fcp0, \
                 tc.tile_pool(name="fcwp", bufs=2) as fcwp0:
                fcp = _RPool(fcp0, rp, reps)
                fcwp = _RPool(fcwp0, rp, reps)
                panel = fcp.tile([128, FT * RWS], BF16, name="panel")
                # m=0 weight panel first so compute can start immediately
                fw0 = fcwp.tile([128, 8192], BF16, name="fcw0", tag="fcw")
                for q4 in range(4):
                    nc.scalar.dma_start(
                        out=fw0[:, q4 * 2048:(q4 + 1) * 2048],
                        in_=fcw_h[0, :, q4 * 2048:(q4 + 1) * 2048])
                # input panel chunked across both queues
                for k in range(FT):
                    eng = nc.sync if k % 2 == 0 else nc.scalar
                    eng.dma_start(out=panel[:, k * RWS:(k + 1) * RWS],
                                  in_=thT_h[k * 128:(k + 1) * 128, :])
                ssq = ssqp.tile([1, 512], F32, name="fcssq", tag="ssq")
                sqs = []
                for m in range(KT):
                    if m == 0:
                        fw = fw0
                    else:
                        fw = fcwp.tile([128, 8192], BF16, name=f"fcw{m}",
                                       tag="fcw")
                        for q4 in range(4):
                            nc.scalar.dma_start(
                                out=fw[:, q4 * 2048:(q4 + 1) * 2048],
                                in_=fcw_h[m, :, q4 * 2048:(q4 + 1) * 2048])
                    ps = mmp.tile([128, RWS], F32, name=f"fcps{m}", tag="mmp")
                    for k in range(FT):
                        nc.tensor.matmul(ps, fw[:, k * 128:(k + 1) * 128],
                                         panel[:, k * RWS:(k + 1) * RWS],
                                         start=(k == 0), stop=(k == FT - 1))
                        # previous m's stats matmul lands mid-k-loop so its
                        # DVE dependency is long done (no PE head-block)
                        if k == 8 and m > 0:
                            nc.tensor.matmul(ssq, ones_bf[:, 0:1], sqs[m - 1],
                                             start=(m == 1), stop=False)
                    sq = temps.tile([128, RWS], BF16, name=f"fcsq{m}",
                                    tag="sq512", bufs=1)
                    # hardware allows only one PSUM read per instruction: the
                    # square runs on the Activation engine (off-loads DVE too)
                    nc.scalar.activation(sq, ps, AF.Square, bias=zb[:, 0:1])
                    sqs.append(sq)
                    t1 = temps.tile([128, RWS], BF16, name=f"fct{m}",
                                    tag="fct", bufs=2)
                    nc.vector.tensor_scalar_mul(t1, ps, hnw[:, m:m + 1])
                    half, mh = divmod(m, KT // 2)
                    nc.sync.dma_start(
                        out=th_loc2[half][mh * 128:(mh + 1) * 128, :], in_=t1)
                    if m == KT // 2 - 1:
                        coll("AllGather", ALU.bypass, th_loc2[0], th_all2[0])
                nc.tensor.matmul(ssq, ones_bf[:, 0:1], sqs[KT - 1],
                                 start=False, stop=True)
                coll("AllGather", ALU.bypass, th_loc2[1], th_all2[1])
                # rstd sideband: rms of raw fc output per token, allgathered
                nc.scalar.activation(ssq, ssq, AF.Sqrt, bias=epsb[:, 0:1],
                                     scale=1.0 / H)
                rms_sb = stats.tile([1, RWS], F32, name="fcrms", tag="rs2")
                nc.vector.tensor_copy(rms_sb, ssq)
                nc.sync.dma_start(out=rl_loc, in_=rms_sb)
                coll("AllGather", ALU.bypass, rl_loc, rl_all)

            if rp == 0:
                # table loads land behind the fc weight stream (read-only,
                # loaded once)
                nc.scalar.dma_start(out=csq, in_=csq_h.ap())
                nc.scalar.dma_start(out=csqn, in_=csqn_h.ap())
                nc.scalar.dma_start(out=ln1, in_=ln1_h.ap())
                nc.scalar.dma_start(out=ln2, in_=ln2_h.ap())
                nc.scalar.dma_start(out=fnw, in_=fnw_h.ap())
                nc.scalar.dma_start(out=qnw, in_=qnw_h.ap())
                nc.scalar.dma_start(out=knw, in_=knw_h.ap())
            nc.scalar.dma_start(
                out=hT.rearrange("p (k n) -> p k n", k=KT),
                in_=hT0_h.ap().rearrange("(k p) n -> p k n", p=128))

            # ---------------- phase 2: big persistent SBUF ----------------
            if rp == 0:
                persist["big"] = ctx.enter_context(
                    tc.tile_pool(name="big", bufs=1))
                persist["wqkv"] = ctx.enter_context(
                    tc.tile_pool(name="wqkv", bufs=7))
                persist["wwop"] = ctx.enter_context(
                    tc.tile_pool(name="wwop", bufs=3))
                persist["wdp"] = ctx.enter_context(
                    tc.tile_pool(name="wdp", bufs=6))
                persist["attp"] = ctx.enter_context(
                    tc.tile_pool(name="attp", bufs=3))
                persist["mid"] = ctx.enter_context(
                    tc.tile_pool(name="mid", bufs=2))
                persist["arup"] = ctx.enter_context(
                    tc.tile_pool(name="arup", bufs=1))
            big = _RPool(persist["big"], rp, reps)
            wqkv = _RPool(persist["wqkv"], rp, reps)
            wwop = _RPool(persist["wwop"], rp, reps)
            wdp = _RPool(persist["wdp"], rp, reps)
            attp = _RPool(persist["attp"], rp, reps)
            mid = _RPool(persist["mid"], rp, reps)
            arup = _RPool(persist["arup"], rp, reps)

            kc = big.tile([128, COLS], BF16, name="kc")
            vrm = [big.tile([128, 17 * 128], BF16, name=f"vrm{b}")
                   for b in range(B)]
            # layer-0 K/V weights load ahead of the th spread on the scalar
            # queue: the first K projection needs them ~3us after fc ends
            wks0 = wqkv.tile([128, 2048], BF16, name="wks_l0", tag="wkv",
                             bufs=2)
            nc.scalar.dma_start(out=wks0, in_=wk_h[0])
            wvs0 = wqkv.tile([128, 2048], BF16, name="wvs_l0", tag="wkv",
                             bufs=2)
            nc.scalar.dma_start(out=wvs0, in_=wv_h[0])
            # th panel on top of the pool stack: closes after the last K/V
            # prefetch so layer 3's whole weight set preloads into the space
            thsb_cm = tc.tile_pool(name="thsbP", bufs=1)
            thsbP = _RPool(thsb_cm.__enter__(), rp, reps)
            thsbA = thsbP.tile([128, HK * B * CTX], BF16, name="thsbA")
            thsbA3 = thsbA.rearrange("p (k c) -> p k c", k=HK)
            thsbB = thsbP.tile([128, HK * B * CTX], BF16, name="thsbB")
            thsbB3 = thsbB.rearrange("p (k c) -> p k c", k=HK)
            spread_half(0, thsbA3)
            spread_half(1, thsbB3)
            thsb = ([thsbA3[:, k, :] for k in range(HK)]
                    + [thsbB3[:, k, :] for k in range(HK)])

            # rstd-derived columns for the softmax trick: col = b*16 + T
            rst = pre.tile([128, 32], F32, name="rst")
            nc.sync.dma_start(out=rst,
                              in_=rl_all.rearrange("r (t p) -> p (r t)",
                                                   p=128))
            irb = pre.tile([128, 32], BF16, name="irb")   # 1/rstd = rms
            nc.vector.tensor_copy(irb, rst)
            lrs = pre.tile([128, 32], F32, name="lrs")    # ln(rstd)=-ln(rms)
            nc.scalar.activation(lrs, rst, AF.Ln, bias=zb[:, 0:1])
            nc.vector.tensor_scalar_mul(lrs, lrs, -1.0)

            # ----- per-layer building blocks -----
            def hnorm(lw_ap, out_bf, nm):
                """out = rms_norm(h) * lnw  -> [128, KT*XC]"""
                sqb = temps.tile([128, KT * XC], BF16, name=f"sqb_{nm}",
                                 tag="sq512", bufs=1)
                for c in range(4):
                    sl = slice(c * 4 * XC, (c + 1) * 4 * XC)
                    nc.vector.tensor_mul(sqb[:, sl], hT[:, sl], hT[:, sl])
                ssq = ssqp.tile([1, 512], F32, name=f"hssq_{nm}", tag="ssq")
                for k in range(KT):
                    nc.tensor.matmul(ssq[:, :XC], ones_bf[:, 0:1],
                                     sqb[:, k * XC:(k + 1) * XC],
                                     start=(k == 0), stop=(k == KT - 1))
                nc.scalar.activation(ssq[:, :XC], ssq[:, :XC], AF.Sqrt,
                                     bias=epsb[:, 0:1], scale=1.0 / H)
                rc = stats.tile([1, XC], F32, name=f"hrc_{nm}", tag="rs2")
                nc.vector.reciprocal(rc, ssq[:, :XC])
                rb = temps.tile([128, XC], F32, name=f"hrb_{nm}", tag="rstdb",
                                bufs=1)
                nc.gpsimd.partition_broadcast(rb, rc)
                h3 = hT.rearrange("p (k n) -> p k n", k=KT)
                o3 = out_bf.rearrange("p (k n) -> p k n", k=KT)
                rb_b = bass.AP(tensor=rb.tensor, offset=rb.offset,
                               ap=[rb.ap[0], [0, 4], rb.ap[1]])
                for c in range(4):
                    ks = slice(c * 4, (c + 1) * 4)
                    ln_c = lw_ap[:, ks]
                    ln_b = bass.AP(tensor=ln_c.tensor, offset=ln_c.offset,
                                   ap=[ln_c.ap[0], ln_c.ap[1], [0, XC]])
                    nc.vector.tensor_tensor(out=o3[:, ks, :],
                                            in0=h3[:, ks, :],
                                            in1=rb_b, op=ALU.mult)
                    nc.vector.tensor_tensor(out=o3[:, ks, :],
                                            in0=o3[:, ks, :],
                                            in1=ln_b, op=ALU.mult)

            def kv_tile(l, b, j, wks, wvs, nm):
                off, w = _bcol(b, j)

                def rhs(k):
                    if j < 4:
                        return thsb[k][:, b * CTX + j * 512:
                                       b * CTX + j * 512 + w]
                    return xT[:, k * XC + b * Q: k * XC + b * Q + w]

                # K projection, then V immediately: the K-norm stats matmul
                # would otherwise head-block the in-order PE queue
                ps = mmp.tile([128, w], F32, name=f"kps_{nm}", tag="mmp")
                for k in range(KT):
                    nc.tensor.matmul(ps, wks[:, k * 128:(k + 1) * 128],
                                     rhs(k),
                                     start=(k == 0), stop=(k == KT - 1))
                ps2 = mmp.tile([128, w], F32, name=f"vps_{nm}", tag="mmp")
                for k in range(KT):
                    nc.tensor.matmul(ps2, wvs[:, k * 128:(k + 1) * 128],
                                     rhs(k),
                                     start=(k == 0), stop=(k == KT - 1))
                kraw = temps.tile([128, w], BF16, name=f"kraw_{nm}",
                                  tag="kraw", bufs=1)
                nc.vector.tensor_copy(kraw, ps)
                rb = rms_bcast([kraw], w, HD, f"kn_{nm}")
                k1 = temps.tile([128, w], BF16, name=f"k1_{nm}", tag="k1",
                                bufs=1)
                nc.vector.tensor_mul(k1, kraw, rb)
                nc.vector.tensor_scalar_mul(k1, k1, knw[:, l:l + 1])
                cst = temps.tile([128, w], BF16, name=f"cs_{nm}", tag="cst",
                                 bufs=2)
                nc.sync.dma_start(out=cst, in_=csk_h[:, off:off + w])
                snt = temps.tile([128, w], BF16, name=f"sn_{nm}", tag="snt",
                                 bufs=2)
                nc.sync.dma_start(out=snt, in_=csn_h[:, off:off + w])
                rope(k1, kc[:, off:off + w], cst, snt, nm)
                vtmp = temps.tile([128, w], BF16, name=f"vtmp_{nm}",
                                  tag="vtmp", bufs=1)
                nc.vector.tensor_copy(vtmp, ps2)
                nch = 4 if j < 4 else 1
                for t in range(nch):
                    cw = 128 if j < 4 else w
                    Tg = j * 4 + t if j < 4 else 16
                    tp = scp.tile([128, 128], BF16, name=f"vtp_{nm}_{t}",
                                  tag="sc")
                    nc.tensor.transpose(tp[0:cw, :],
                                        vtmp[:, t * 128:t * 128 + cw], ident)
                    nc.vector.tensor_copy(
                        vrm[b][0:cw, Tg * 128:(Tg + 1) * 128], tp[0:cw, :])

            def kvw_load(l, nm):
                wks = wqkv.tile([128, 2048], BF16, name=f"wks_{nm}",
                                tag="wkv", bufs=2)
                nc.scalar.dma_start(out=wks, in_=wk_h[l])
                wvs = wqkv.tile([128, 2048], BF16, name=f"wvs_{nm}",
                                tag="wkv", bufs=2)
                nc.scalar.dma_start(out=wvs, in_=wv_h[l])
                return wks, wvs

            def kv_half(l, b, wks, wvs, nm):
                for j in range(4):
                    kv_tile(l, b, j, wks, wvs, f"{nm}_{b}_{j}")

            xT = mid.tile([128, KT * XC], BF16, name="xT_init", tag="xT",
                          bufs=1)
            interT = mid.tile([128, IT * XC], BF16, name="inter_init",
                              tag="inter", bufs=1)
            aru = arup.tile([128, KT * XC], F32, name="aru", tag="aru")

            # layer-0 ctx K/V runs as soon as th lands
            kvw_next = (wks0, wvs0)
            kv_half(0, 0, *kvw_next, "l0")
            kv_half(0, 1, *kvw_next, "l0")

            for l in range(L):
                nm = f"L{l}"
                if l == L - 1:
                    # thsb is dead (closed at the end of l==2); preload the
                    # whole layer-3 weight set so its MLP is never DMA-
                    # throttled and the stream overlaps attention + both ARs
                    l3w_cm = tc.tile_pool(name="l3w", bufs=1)
                    l3w = _RPool(l3w_cm.__enter__(), rp, reps)
                    wosl = []
                    for m in range(KT):
                        wos = l3w.tile([128, 256], BF16, name=f"l3wo{m}")
                        nc.scalar.dma_start(out=wos, in_=wo_h[l, m])
                        wosl.append(wos)
                    gup = []
                    for m in range(IT):
                        ws = []
                        for h2 in range(2):
                            g2 = l3w.tile([128, 1024], BF16,
                                          name=f"l3g{m}_{h2}")
                            nc.scalar.dma_start(
                                out=g2,
                                in_=gw_h[l, m, :, h2 * 1024:(h2 + 1) * 1024])
                            u2 = l3w.tile([128, 1024], BF16,
                                          name=f"l3u{m}_{h2}")
                            nc.scalar.dma_start(
                                out=u2,
                                in_=uw_h[l, m, :, h2 * 1024:(h2 + 1) * 1024])
                            ws.append((g2, u2))
                        gup.append(ws)
                    dwn = []
                    for m in range(KT):
                        dws = l3w.tile([128, 768], BF16, name=f"l3d{m}")
                        nc.scalar.dma_start(out=dws, in_=dw_h[l, m])
                        dwn.append(dws)
                # x = rms_norm(h, ln1)
                hnorm(ln1[:, l * KT:(l + 1) * KT], xT, f"x1_{nm}")
                # q projection, both heads batched through one norm+rope pass
                qcat = temps.tile([128, 2 * XC], BF16, name=f"qraw_{nm}",
                                  tag="kraw", bufs=1)
                for hh in range(2):
                    wqs = []
                    for h2 in range(2):
                        wq2 = wqkv.tile([128, 1024], BF16,
                                        name=f"wqs_{nm}{hh}_{h2}", tag="wqkv")
                        nc.scalar.dma_start(
                            out=wq2,
                            in_=wq_h[l, hh, :, h2 * 1024:(h2 + 1) * 1024])
                        wqs.append(wq2)
                    ps = mm64.tile([128, XC], F32, name=f"qps_{nm}{hh}",
                                   tag="mm64")
                    for k in range(KT):
                        nc.tensor.matmul(ps, wqs[k // 8][:, (k % 8) * 128:
                                                         (k % 8 + 1) * 128],
                                         xT[:, k * XC:(k + 1) * XC],
                                         start=(k == 0), stop=(k == KT - 1))
                    nc.vector.tensor_copy(qcat[:, hh * XC:(hh + 1) * XC], ps)
                rb = rms_bcast([qcat], 2 * XC, HD, f"qn_{nm}")
                q1 = temps.tile([128, 2 * XC], BF16, name=f"q1_{nm}",
                                tag="k1", bufs=1)
                nc.vector.tensor_mul(q1, qcat, rb)
                nc.vector.tensor_scalar_mul(q1, q1, qnw[:, l:l + 1])
                qq = attp.tile([128, 2 * XC], BF16, name=f"qro_{nm}",
                               tag="qro0", bufs=2)
                csq_b = bass.AP(tensor=csq.tensor, offset=csq.offset,
                                ap=[csq.ap[0], [0, 2], csq.ap[1]])
                csqn_b = bass.AP(tensor=csqn.tensor, offset=csqn.offset,
                                 ap=[csqn.ap[0], [0, 2], csqn.ap[1]])
                rope(q1, qq, csq_b, csqn_b, f"q_{nm}")
                qro = [qq[:, 0:XC], qq[:, XC:2 * XC]]
                # tail kv tiles (depend on x)
                wks, wvs = kvw_next
                for b in range(B):
                    kv_tile(l, b, 4, wks, wvs, f"t_{nm}_{b}")
                # prefetch wo panels during attention
                if l < L - 1:
                    wosl = []
                    for m in range(KT):
                        wos = wwop.tile([128, 256], BF16, name=f"wos_{nm}{m}",
                                        tag="wwo")
                        nc.scalar.dma_start(out=wos, in_=wo_h[l, m])
                        wosl.append(wos)
                # attention: both q heads share the kv head
                o_h = [attp.tile([128, XC], BF16, name=f"oh_{nm}{hh}",
                                 tag=f"oh{hh}", bufs=1) for hh in range(2)]
                for b in range(B):
                    ssum = mm64.tile([1, XC], F32, name=f"ssum_{nm}{b}",
                                     tag="mm64")
                    oT = [mm64.tile([128, Q], F32, name=f"oT_{nm}{b}{hh}",
                                    tag="mm64") for hh in range(2)]
                    nt = 17

                    def acc(T, ex, cnt, scol):
                        nc.tensor.matmul(ssum, scol, ex[0:cnt, :],
                                         start=(T == 0), stop=(T == nt - 1))
                        for hh in range(2):
                            nc.tensor.matmul(
                                oT[hh],
                                vrm[b][0:cnt, T * 128:(T + 1) * 128],
                                ex[0:cnt, hh * Q:(hh + 1) * Q],
                                start=(T == 0), stop=(T == nt - 1))

                    # software-pipelined: tile T's accumulation matmuls land
                    # after tile T+1's score matmuls so the Exp latency never
                    # head-blocks the in-order PE queue
                    pend = None
                    for T in range(nt):
                        cnt = 128 if T < 16 else KV - CTX
                        koff = b * KV + T * 128
                        sc = scp.tile([128, XC], F32, name=f"sc_{nm}{b}{T}",
                                      tag="sc")
                        for hh in range(2):
                            nc.tensor.matmul(sc[0:cnt, hh * Q:(hh + 1) * Q],
                                             kc[:, koff:koff + cnt],
                                             qro[hh][:, b * Q:(b + 1) * Q],
                                             start=True, stop=True)
                        ex = attp.tile([128, XC], BF16, name=f"ex_{nm}{b}{T}",
                                       tag="exps")
                        # ctx tiles: exp bias ln(rstd) folds the deferred
                        # hidden-norm rstd into the V path; denominator uses
                        # a 1/rstd column instead of ones.
                        if T < 16:
                            cidx = b * 16 + T
                            ebias = lrs[0:cnt, cidx:cidx + 1]
                            scol = irb[0:cnt, cidx:cidx + 1]
                        else:
                            ebias = zb[0:cnt, 0:1]
                            scol = ones_bf[0:cnt, 0:1]
                        nc.scalar.activation(ex[0:cnt, :], sc[0:cnt, :],
                                             AF.Exp, bias=ebias, scale=SCALE)
                        if pend is not None:
                            acc(*pend)
                        pend = (T, ex, cnt, scol)
                    acc(*pend)
                    rc = stats.tile([1, XC], F32, name=f"orc_{nm}{b}",
                                    tag="rs2")
                    nc.vector.reciprocal(rc, ssum)
                    rb = temps.tile([128, XC], F32, name=f"orb_{nm}{b}",
                                    tag="rstdb", bufs=1)
                    nc.gpsimd.partition_broadcast(rb, rc)
                    for hh in range(2):
                        nc.vector.tensor_mul(o_h[hh][:, b * Q:(b + 1) * Q],
                                             oT[hh],
                                             rb[:, hh * Q:(hh + 1) * Q])
                # wo projection -> partial h update -> AllReduce.  h/8 staged
                # up front (the AllReduce of partial + h/8 yields NEW h).
                nc.vector.tensor_scalar_mul(aru, hT, 0.125)
                for m in range(KT):
                    wos = wosl[m]
                    wop = mm64.tile([128, XC], F32, name=f"wop_{nm}{m}",
                                    tag="mm64")
                    for kh in range(2):
                        nc.tensor.matmul(wop,
                                         wos[:, kh * 128:(kh + 1) * 128],
                                         o_h[kh], start=(kh == 0),
                                         stop=(kh == 1))
                    asl = aru[:, m * XC:(m + 1) * XC]
                    nc.vector.tensor_add(asl, asl, wop)
                ar_in = arp.tile([H, XC], F32, name=f"ari_{nm}a", tag="arin")
                ar_out = arp.tile([H, XC], F32, name=f"aro_{nm}a",
                                  tag="arout", addr_space="Shared")
                nc.sync.dma_start(
                    out=ar_in.rearrange("(k p) n -> p k n", p=128),
                    in_=aru.rearrange("p (k n) -> p k n", k=KT))
                coll("AllReduce", ALU.add, ar_in, ar_out)
                # next layer ctx K/V (batch 0) fills the AllReduce gap
                if l + 1 < L:
                    kvw_next = kvw_load(l + 1, f"l{l + 1}")
                    kv_half(l + 1, 0, *kvw_next, f"l{l + 1}")
                # prefetch MLP weight panels (independent of the AllReduce)
                if l < L - 1:
                    gup = []
                    for m in range(IT):
                        ws = []
                        for h2 in range(2):
                            g2 = wqkv.tile([128, 1024], BF16,
                                           name=f"gws_{nm}{m}_{h2}",
                                           tag="wqkv")
                            nc.scalar.dma_start(
                                out=g2,
                                in_=gw_h[l, m, :, h2 * 1024:(h2 + 1) * 1024])
                            u2 = wqkv.tile([128, 1024], BF16,
                                           name=f"uws_{nm}{m}_{h2}",
                                           tag="wqkv")
                            nc.scalar.dma_start(
                                out=u2,
                                in_=uw_h[l, m, :, h2 * 1024:(h2 + 1) * 1024])
                            ws.append((g2, u2))
                        gup.append(ws)
                    dwn = []
                    for m in range(KT):
                        dws = wdp.tile([128, 768], BF16, name=f"dws_{nm}{m}",
                                       tag="wdn")
                        nc.scalar.dma_start(out=dws, in_=dw_h[l, m])
                        dwn.append(dws)
                # chunked readback: hnorm starts on the first quarter of h
                h4 = hT.rearrange("p (k n) -> p k n", k=KT)
                a4 = ar_out.rearrange("(k p) n -> p k n", p=128)
                for c in range(4):
                    ks = slice(c * 4, (c + 1) * 4)
                    nc.sync.dma_start(out=h4[:, ks, :], in_=a4[:, ks, :])
                # MLP (x2 reuses the xT tile)
                hnorm(ln2[:, l * KT:(l + 1) * KT], xT, f"x2_{nm}")
                for m in range(IT):
                    gps = mm64.tile([128, XC], F32, name=f"gps_{nm}{m}",
                                    tag="mm64")
                    for k in range(KT):
                        nc.tensor.matmul(
                            gps,
                            gup[m][k // 8][0][:, (k % 8) * 128:
                                              (k % 8 + 1) * 128],
                            xT[:, k * XC:(k + 1) * XC],
                            start=(k == 0), stop=(k == KT - 1))
                    ups = mm64.tile([128, XC], F32, name=f"ups_{nm}{m}",
                                    tag="mm64")
                    for k in range(KT):
                        nc.tensor.matmul(
                            ups,
                            gup[m][k // 8][1][:, (k % 8) * 128:
                                              (k % 8 + 1) * 128],
                            xT[:, k * XC:(k + 1) * XC],
                            start=(k == 0), stop=(k == KT - 1))
                    sil = temps.tile([128, XC], BF16, name=f"sil_{nm}{m}",
                                     tag="kraw", bufs=1)
                    nc.scalar.activation(sil, gps, AF.Silu, bias=zb[:, 0:1])
                    nc.vector.tensor_mul(interT[:, m * XC:(m + 1) * XC],
                                         sil, ups)
                nc.vector.tensor_scalar_mul(aru, hT, 0.125)
                for m in range(KT):
                    dws = dwn[m]
                    dps = mm64.tile([128, XC], F32, name=f"dps_{nm}{m}",
                                    tag="mm64")
                    for k in range(IT):
                        nc.tensor.matmul(dps, dws[:, k * 128:(k + 1) * 128],
                                         interT[:, k * XC:(k + 1) * XC],
                                         start=(k == 0), stop=(k == IT - 1))
                    asl2 = aru[:, m * XC:(m + 1) * XC]
                    nc.vector.tensor_add(asl2, asl2, dps)
                ar_in2 = arp.tile([H, XC], F32, name=f"ari_{nm}b", tag="arin")
                ar_out2 = arp.tile([H, XC], F32, name=f"aro_{nm}b",
                                   tag="arout", addr_space="Shared")
                nc.sync.dma_start(
                    out=ar_in2.rearrange("(k p) n -> p k n", p=128),
                    in_=aru.rearrange("p (k n) -> p k n", k=KT))
                coll("AllReduce", ALU.add, ar_in2, ar_out2)
                # next layer ctx K/V (batch 1) fills the second AllReduce
                # gap; issued before the readback so the sync queue isn't
                # head-blocked on the collective semaphore
                if l + 1 < L:
                    kv_half(l + 1, 1, *kvw_next, f"l{l + 1}")
                    if l + 1 == L - 1:
                        thsb_cm.__exit__(None, None, None)
                h4b = hT.rearrange("p (k n) -> p k n", k=KT)
                a4b = ar_out2.rearrange("(k p) n -> p k n", p=128)
                for c in range(4):
                    ks = slice(c * 4, (c + 1) * 4)
                    nc.sync.dma_start(out=h4b[:, ks, :], in_=a4b[:, ks, :])

            l3w_cm.__exit__(None, None, None)
            # final norm -> outT (chunked so output DMAs start early)
            fin = arup.tile([128, KT * XC], F32, name="fin", tag="aru")
            hnorm(fnw, fin, "fin")
            o4 = outT_h.ap().rearrange("(k p) n -> p k n", p=128)
            f4 = fin.rearrange("p (k n) -> p k n", k=KT)
            for c in range(4):
                ks = slice(c * 4, (c + 1) * 4)
                nc.sync.dma_start(out=o4[:, ks, :], in_=f4[:, ks, :])

        for rp in range(reps):
            body(rp)

    nc.compile()
    return nc


def _prep_inputs(inputs):
    ne = np.asarray(inputs["noise_embedding"], np.float32)
    th = np.asarray(inputs["target_hidden"], np.float32)
    pos = np.asarray(inputs["position_ids"])
    fc = np.asarray(inputs["fc_w"], np.float32)
    wq = np.asarray(inputs["wq"], np.float32)
    wk = np.asarray(inputs["wk"], np.float32)
    wv = np.asarray(inputs["wv"], np.float32)
    wo = np.asarray(inputs["wo"], np.float32)
    gw = np.asarray(inputs["gate_w"], np.float32)
    uw = np.asarray(inputs["up_w"], np.float32)
    dw = np.asarray(inputs["down_w"], np.float32)

    fcw_t = np.ascontiguousarray(
        fc.reshape(64, 128, 16, 128).transpose(2, 1, 0, 3)
    ).reshape(16, 128, 8192).astype(BF)
    hT0 = np.ascontiguousarray(ne.reshape(XC, H).T).astype(np.float32)

    inv = 1.0 / (THETA ** (np.arange(0, HD, 2, dtype=np.float32) / HD))
    ang = pos.astype(np.float32)[:, :, None] * inv[None, None, :]  # [B,KV,64]
    # cos/sin duplicated across both 64-partition halves
    csk = np.empty((128, COLS), np.float32)
    csn = np.empty((128, COLS), np.float32)
    csq = np.empty((128, XC), np.float32)
    csqn = np.empty((128, XC), np.float32)
    for b in range(B):
        ck, sk = np.cos(ang[b]).T, np.sin(ang[b]).T
        csk[0:64, b * KV:(b + 1) * KV] = ck
        csk[64:128, b * KV:(b + 1) * KV] = ck
        csn[0:64, b * KV:(b + 1) * KV] = sk
        csn[64:128, b * KV:(b + 1) * KV] = sk
        cq, sq = np.cos(ang[b, KV - Q:]).T, np.sin(ang[b, KV - Q:]).T
        csq[0:64, b * Q:(b + 1) * Q] = cq
        csq[64:128, b * Q:(b + 1) * Q] = cq
        csqn[0:64, b * Q:(b + 1) * Q] = sq
        csqn[64:128, b * Q:(b + 1) * Q] = sq

    ln1w = np.ascontiguousarray(
        np.asarray(inputs["ln1_w"], np.float32).reshape(L, KT, 128)
        .transpose(2, 0, 1)).reshape(128, L * KT)
    ln2w = np.ascontiguousarray(
        np.asarray(inputs["ln2_w"], np.float32).reshape(L, KT, 128)
        .transpose(2, 0, 1)).reshape(128, L * KT)
    hnw = np.ascontiguousarray(
        np.asarray(inputs["hidden_norm_w"], np.float32).reshape(KT, 128).T)
    fnw = np.ascontiguousarray(
        np.asarray(inputs["final_norm_w"], np.float32).reshape(KT, 128).T)
    qnw = np.ascontiguousarray(np.asarray(inputs["qn_w"], np.float32).T)
    knw = np.ascontiguousarray(np.asarray(inputs["kn_w"], np.float32).T)

    flat = th.reshape(B * CTX, 8192)
    in_maps = []
    for c in range(NCORES):
        thT_c = np.ascontiguousarray(
            flat[c * RWS:(c + 1) * RWS].T).astype(BF)
        wq_c = np.ascontiguousarray(
            wq[:, :, c * 256:(c + 1) * 256]
            .reshape(L, 16, 128, 2, 128).transpose(0, 3, 2, 1, 4)
        ).reshape(L, 2, 128, 2048).astype(BF)
        wk_c = np.ascontiguousarray(
            wk[:, :, c * 128:(c + 1) * 128]
            .reshape(L, 16, 128, 128).transpose(0, 2, 1, 3)
        ).reshape(L, 128, 2048).astype(BF)
        wv_c = np.ascontiguousarray(
            wv[:, :, c * 128:(c + 1) * 128]
            .reshape(L, 16, 128, 128).transpose(0, 2, 1, 3)
        ).reshape(L, 128, 2048).astype(BF)
        wo_c = np.ascontiguousarray(
            wo[:, c * 256:(c + 1) * 256, :]
            .reshape(L, 2, 128, 16, 128).transpose(0, 3, 2, 1, 4)
        ).reshape(L, 16, 128, 256).astype(BF)
        gw_c = np.ascontiguousarray(
            gw[:, :, c * 768:(c + 1) * 768]
            .reshape(L, 16, 128, 6, 128).transpose(0, 3, 2, 1, 4)
        ).reshape(L, 6, 128, 2048).astype(BF)
        uw_c = np.ascontiguousarray(
            uw[:, :, c * 768:(c + 1) * 768]
            .reshape(L, 16, 128, 6, 128).transpose(0, 3, 2, 1, 4)
        ).reshape(L, 6, 128, 2048).astype(BF)
        dw_c = np.ascontiguousarray(
            dw[:, c * 768:(c + 1) * 768, :]
            .reshape(L, 6, 128, 16, 128).transpose(0, 3, 2, 1, 4)
        ).reshape(L, 16, 128, 768).astype(BF)
        in_maps.append(dict(
            thT=thT_c, fcw=fcw_t, hT0=hT0,
            wq=wq_c, wk=wk_c, wv=wv_c, wo=wo_c,
            gw=gw_c, uw=uw_c, dw=dw_c,
            csk=csk.astype(BF), csn=csn.astype(BF),
            csq=csq.astype(BF), csqn=csqn.astype(BF),
            ln1w=ln1w, ln2w=ln2w, hnw=hnw, fnw=fnw, qnw=qnw, knw=knw,
        ))
    return in_maps


_last_results = None


def kernel(**inputs):
    global _last_results
    if "nc" not in _CACHE:
        _CACHE["nc"] = build_program()
    nc = _CACHE["nc"]
    in_maps = _prep_inputs(inputs)
    res = run_bass_kernel_spmd(nc, in_maps, core_ids=list(range(NCORES)),
                               trace=TRACE)
    _last_results = res
    outT = res.results[0]["outT"]
    return np.ascontiguousarray(outT.T).reshape(B, Q, H).astype(np.float32)
